# revision 18
# baseline (speedup 1.0000x reference)
"""Trainium2 Bass kernel for nn_MILModel7 (MIL model, 32 bags x 2048 instances).

Math note: the reference's pairwise block
    pair[b,i,j] = s_v[b,i] + s_u[b,j] + W_b ; p = softmax(pair, axis=i)
has the j-dependent terms constant along the softmax axis, so they cancel:
    p[b,i,j] = softmax(s_v[b,:])[i]  for every j, and
    scores   = p.mean(axis=j)       = softmax(s_v[b,:]).
The O(n^2) pairwise work therefore collapses to a 1-D softmax over s_v.
W_b never affects the output (softmax-shift invariant) and is dropped.

Sharding: data-parallel over bags, 4 bags per core across 8 cores; small
weights replicated. On-chip layout is feature-major ("transposed"): every
activation lives as [feat_partitions, token_free], so each layer is a
stationary-weight matmul streaming 512-token chunks. Inputs are transposed
and cast to fp16 on the host (marshalling); accumulation is fp32 in PSUM,
softmax/statistics are fp32 on-chip.
"""

import sys

try:
    import concourse.bass as bass  # noqa: F401
except ImportError:
    sys.path.insert(0, "/opt/trn_rl_repo")

import numpy as np

import concourse.bass as bass
import concourse.tile as tile
from concourse import bacc, mybir
from concourse.bass_utils import run_bass_kernel_spmd

AFT = mybir.ActivationFunctionType
ALU = mybir.AluOpType
F16 = mybir.dt.float16
F32 = mybir.dt.float32
F32R = mybir.dt.float32r
AX = mybir.AxisListType.X

B = 32          # bags total
NCORES = 8
BPC = B // NCORES   # bags per core
N = 2048        # instances per bag
NC_CH = 4       # token chunks per bag
CH = N // NC_CH  # 512 tokens per chunk
VD = 512        # vfeat dim
KV = VD // 128  # contraction chunks for vfc
AD = 128        # afeat / hidden dim
HD = 64         # V output dim


def _build_program():
    nc = bacc.Bacc("TRN2", target_bir_lowering=False, debug=False)

    dp = nc.dram_tensor
    vfT = dp("vfT", (BPC, KV, 128, N), F16, kind="ExternalInput").ap()
    afT = dp("afT", (BPC, 128, N), F16, kind="ExternalInput").ap()
    w_vfc = dp("w_vfc", (KV, 128, 128), F16, kind="ExternalInput").ap()
    w_afc = dp("w_afc", (128, 128), F16, kind="ExternalInput").ap()
    w_amv = dp("w_amv", (128, 128), F16, kind="ExternalInput").ap()
    w_ama = dp("w_ama", (128, 128), F16, kind="ExternalInput").ap()
    w_amo = dp("w_amo", (128, 1), F16, kind="ExternalInput").ap()
    w_V = dp("w_V", (128, HD), F16, kind="ExternalInput").ap()
    w_W = dp("w_W", (HD, 1), F16, kind="ExternalInput").ap()
    w_cls = dp("w_cls", (128, 2), F32, kind="ExternalInput").ap()
    b_vfc = dp("b_vfc", (128, 1), F32, kind="ExternalInput").ap()
    b_afc = dp("b_afc", (128, 1), F32, kind="ExternalInput").ap()
    b_am = dp("b_am", (128, 1), F32, kind="ExternalInput").ap()
    b_V = dp("b_V", (HD, 1), F32, kind="ExternalInput").ap()
    b_amo = dp("b_amo", (BPC, 1), F32, kind="ExternalInput").ap()
    b_cls = dp("b_cls", (2, 1), F32, kind="ExternalInput").ap()

    ones32 = dp("ones32", (1, 128), F32, kind="ExternalInput").ap()

    scores_out = dp("scores", (BPC, N), F32, kind="ExternalOutput").ap()
    logits_out = dp("logits", (BPC, 2), F32, kind="ExternalOutput").ap()

    with tile.TileContext(nc) as tc:
        with (
            tc.tile_pool(name="wp", bufs=1) as wp,
            tc.tile_pool(name="vfp", bufs=BPC * KV) as vfp,
            tc.tile_pool(name="afp", bufs=BPC) as afp,
            tc.tile_pool(name="vo1p", bufs=5) as vo1p,
            tc.tile_pool(name="axp", bufs=5) as axp,
            tc.tile_pool(name="hp", bufs=4) as hp,
            tc.tile_pool(name="attp", bufs=4) as attp,
            tc.tile_pool(name="ffp", bufs=5) as ffp,
            tc.tile_pool(name="fvp", bufs=4) as fvp,
            tc.tile_pool(name="tmpp", bufs=4) as tmpp,
            tc.tile_pool(name="scrp", bufs=3) as scrp,
            tc.tile_pool(name="mscp", bufs=1) as mscp,
            tc.tile_pool(name="ps_mm", bufs=3, space="PSUM") as ps_mm,
            tc.tile_pool(name="ps_row", bufs=2, space="PSUM") as ps_row,
            tc.tile_pool(name="ps_bc", bufs=3, space="PSUM") as ps_bc,
        ):
            # --- weights / biases / constants to SBUF ---
            vfc_sb = wp.tile([128, KV * 128], F16, tag="vfc")
            for k in range(KV):
                nc.sync.dma_start(vfc_sb[:, k * 128:(k + 1) * 128], w_vfc[k])
            afc_sb = wp.tile([128, 128], F16, tag="afc")
            nc.sync.dma_start(afc_sb[:], w_afc)
            amv_sb = wp.tile([128, 128], F16, tag="amv")
            nc.sync.dma_start(amv_sb[:], w_amv)
            ama_sb = wp.tile([128, 128], F16, tag="ama")
            nc.sync.dma_start(ama_sb[:], w_ama)
            amo_sb = wp.tile([128, 1], F16, tag="amo")
            nc.sync.dma_start(amo_sb[:], w_amo)
            V_sb = wp.tile([128, HD], F16, tag="V")
            nc.sync.dma_start(V_sb[:], w_V)
            W_sb = wp.tile([HD, 1], F16, tag="W")
            nc.sync.dma_start(W_sb[:], w_W)
            cls_sb = wp.tile([128, 2], F32, tag="cls")
            nc.sync.dma_start(cls_sb[:], w_cls)
            vfcb_sb = wp.tile([128, 1], F32, tag="vfcb")
            nc.sync.dma_start(vfcb_sb[:], b_vfc)
            afcb_sb = wp.tile([128, 1], F32, tag="afcb")
            nc.sync.dma_start(afcb_sb[:], b_afc)
            amb_sb = wp.tile([128, 1], F32, tag="amb")
            nc.sync.dma_start(amb_sb[:], b_am)
            Vb_sb = wp.tile([HD, 1], F32, tag="Vb")
            nc.sync.dma_start(Vb_sb[:], b_V)
            amob_sb = wp.tile([BPC, 1], F32, tag="amob")
            nc.sync.dma_start(amob_sb[:], b_amo)
            clsb_sb = wp.tile([2, 1], F32, tag="clsb")
            nc.sync.dma_start(clsb_sb[:], b_cls)
            o16_sb = wp.tile([1, 128], F16, tag="o16")
            nc.vector.memset(o16_sb[:], 1.0)
            o32f_sb = wp.tile([1, 128], F32, tag="o32f")
            nc.sync.dma_start(o32f_sb[:], ones32)
            o32_sb = wp.tile([1, 128], F32R, tag="o32")
            nc.scalar.activation(o32_sb[:], o32f_sb[:], AFT.Copy)

            # --- stream activations in (all bags resident) ---
            vf_sb = [[vfp.tile([128, N], F16, tag="vf", name=f"vf{b}_{k}")
                      for k in range(KV)] for b in range(BPC)]
            af_sb = [afp.tile([128, N], F16, tag="af", name=f"af{b}")
                     for b in range(BPC)]
            qs = [nc.sync, nc.scalar, nc.gpsimd]
            qi = 0
            for b in range(BPC):
                for k in range(KV):
                    qs[qi % len(qs)].dma_start(vf_sb[b][k][:], vfT[b, k])
                    qi += 1
                qs[qi % len(qs)].dma_start(af_sb[b][:], afT[b])
                qi += 1

            # --- persistent stats tiles (per bag, partition 0: PE rhs and
            # lhsT operands must sit at base partition 0/32/64) ---
            e_sb = [mscp.tile([1, N], F32R, tag=f"e{b}", name=f"e{b}")
                    for b in range(BPC)]
            zacc = mscp.tile([128, BPC * NC_CH], F32, tag="zacc")  # zfeat partials

            for c in range(NC_CH):
                cs = slice(c * CH, (c + 1) * CH)
                h_c = {}
                ax_c = {}
                vo1_c = {}
                att_c = {}
                for b in range(BPC):
                    # vo1 = relu(vfc_w @ vfeat + b)
                    p_vo1 = ps_mm.tile([128, CH], F32, tag="mm")
                    for k in range(KV):
                        nc.tensor.matmul(p_vo1[:], vfc_sb[:, k * 128:(k + 1) * 128],
                                         vf_sb[b][k][:, cs],
                                         start=(k == 0), stop=(k == KV - 1))
                    vo1_c[b] = vo1p.tile([128, CH], F16, tag="vo1", name=f"vo1_{c}_{b}")
                    nc.scalar.activation(vo1_c[b][:], p_vo1[:], AFT.Relu,
                                         bias=vfcb_sb[:])
                    # ax = relu(afc_w @ afeat + b)
                    p_ax = ps_mm.tile([128, CH], F32, tag="mm")
                    nc.tensor.matmul(p_ax[:], afc_sb[:], af_sb[b][:, cs],
                                     start=True, stop=True)
                    ax_c[b] = axp.tile([128, CH], F16, tag="ax", name=f"ax_{c}_{b}")
                    nc.vector.tensor_scalar(ax_c[b][:], p_ax[:], afcb_sb[:], 0.0,
                                            ALU.add, ALU.max)
                    # h = tanh(amv @ vo1 + ama @ ax + b)
                    p_h = ps_mm.tile([128, CH], F32, tag="mm")
                    nc.tensor.matmul(p_h[:], amv_sb[:], vo1_c[b][:],
                                     start=True, stop=False)
                    nc.tensor.matmul(p_h[:], ama_sb[:], ax_c[b][:],
                                     start=False, stop=True)
                    h_c[b] = hp.tile([128, CH], F16, tag="h", name=f"h_{c}_{b}")
                    nc.scalar.activation(h_c[b][:], p_h[:], AFT.Tanh,
                                         bias=amb_sb[:])
                    # att_pre = am_o_w @ h ; att = sigmoid(att_pre + am_o_b)
                    p_att = ps_row.tile([1, CH], F32, tag="rowmm",
                                        name=f"p_att_{c}_{b}")
                    nc.tensor.matmul(p_att[:], amo_sb[:], h_c[b][:],
                                     start=True, stop=True)
                    att_c[b] = attp.tile([1, CH], F16, tag="att",
                                         name=f"att_{c}_{b}")
                    nc.scalar.activation(att_c[b][:], p_att[:], AFT.Sigmoid,
                                         bias=amob_sb[0:1, :])

                ff_c = {}
                for b in range(BPC):
                    # broadcast att row to 128 partitions via PE
                    p_a128 = ps_bc.tile([128, CH], F32, tag="bc")
                    nc.tensor.matmul(p_a128[:], o16_sb[:], att_c[b][:],
                                     start=True, stop=True)
                    # ffeat = att * ax + vo1
                    t_c = tmpp.tile([128, CH], F16, tag="tmp")
                    nc.vector.tensor_tensor(t_c[:], p_a128[:], ax_c[b][:], ALU.mult)
                    ff_c[b] = ffp.tile([128, CH], F16, tag="ff", name=f"ff_{c}_{b}")
                    nc.gpsimd.tensor_tensor(ff_c[b][:], t_c[:], vo1_c[b][:],
                                            ALU.add)
                    # f_v = relu(V @ ffeat + b)
                    p_fv = ps_mm.tile([HD, CH], F32, tag="mm")
                    nc.tensor.matmul(p_fv[:], V_sb[:], ff_c[b][:],
                                     start=True, stop=True)
                    fv_c = fvp.tile([HD, CH], F16, tag="fv")
                    nc.vector.tensor_scalar(fv_c[:], p_fv[:], Vb_sb[:], 0.0,
                                            ALU.add, ALU.max)
                    # s_v = W @ f_v ; e = exp(s_v) + row-sum partial (no max
                    # shift: |s_v| stays far below fp32 exp overflow)
                    p_sv = ps_row.tile([1, CH], F32, tag="rowmm",
                                       name=f"p_sv_{c}_{b}")
                    nc.tensor.matmul(p_sv[:], W_sb[:], fv_c[:],
                                     start=True, stop=True)
                    nc.scalar.activation(e_sb[b][:, cs], p_sv[:], AFT.Exp)
                for b in range(BPC):
                    # broadcast e row, weighted-accumulate ffeat into zacc
                    p_e128 = ps_bc.tile([128, CH], F32, tag="bc")
                    nc.tensor.matmul(p_e128[:], o32_sb[:], e_sb[b][:, cs],
                                     start=True, stop=True)
                    scr = scrp.tile([128, CH], F32, tag="scr")
                    col = b * NC_CH + c
                    nc.vector.tensor_tensor(scr[:], p_e128[:], ff_c[b][:],
                                            ALU.mult)
                    nc.vector.reduce_sum(zacc[:, col:col + 1], scr[:], axis=AX)

            # --- tail: normalize, classify ---
            invZ24 = mscp.tile([2, BPC], F32, tag="invZ24")
            for b in range(BPC):
                Z_b = mscp.tile([1, 1], F32, tag=f"Z{b}", name=f"Z{b}")
                nc.vector.reduce_sum(Z_b[:], e_sb[b][:].bitcast(F32), axis=AX)
                iz = mscp.tile([1, 1], F32, tag=f"invZ{b}", name=f"invZ{b}")
                nc.vector.reciprocal(iz[:], Z_b[:])
                sc_b = mscp.tile([1, N], F32, tag=f"sc{b}", name=f"sc{b}")
                nc.gpsimd.tensor_scalar(sc_b[:], e_sb[b][:].bitcast(F32), iz[:],
                                        None, ALU.mult)
                nc.sync.dma_start(scores_out[b:b + 1, :], sc_b[:])
                # replicate Z to two partitions via PE, reciprocal into col b
                p_z2 = ps_row.tile([2, 1], F32, tag="rowmm", name=f"p_z2_{b}")
                nc.tensor.matmul(p_z2[:], o32_sb[0:1, 0:2].bitcast(F32),
                                 Z_b[:], start=True, stop=True)
                nc.vector.reciprocal(invZ24[:, b:b + 1], p_z2[:])

            p_lg = ps_row.tile([2, BPC], F32, tag="rowmm")
            for b in range(BPC):
                zun = mscp.tile([128, 1], F32, tag=f"zun{b}")
                nc.vector.reduce_sum(zun[:], zacc[:, b * NC_CH:(b + 1) * NC_CH],
                                     axis=AX)
                nc.tensor.matmul(p_lg[:, b:b + 1], cls_sb[:], zun[:],
                                 start=True, stop=True)
            lg1 = mscp.tile([2, BPC], F32, tag="lg1")
            nc.vector.tensor_tensor(lg1[:], p_lg[:], invZ24[:], ALU.mult)
            logits_sb = mscp.tile([2, BPC], F32, tag="lgsb")
            nc.vector.tensor_scalar(logits_sb[:], lg1[:], clsb_sb[:], None, ALU.add)
            for b in range(BPC):
                nc.sync.dma_start(logits_out[b:b + 1, :], logits_sb[:, b:b + 1])

    nc.compile()
    return nc


_NC = None


def _get_program():
    global _NC
    if _NC is None:
        _NC = _build_program()
    return _NC


def _prep_inputs(inputs):
    f16 = np.float16
    f32 = np.float32
    g = {k: np.asarray(v, f32) for k, v in inputs.items()}
    vfT_all = np.ascontiguousarray(g["vfeat"].transpose(0, 2, 1)).astype(f16)
    vfT_all = vfT_all.reshape(B, KV, 128, N)
    afT_all = np.ascontiguousarray(g["afeat"].transpose(0, 2, 1)).astype(f16)

    shared = {
        "w_vfc": np.ascontiguousarray(g["vfc_w"].T).astype(f16).reshape(KV, 128, 128),
        "w_afc": np.ascontiguousarray(g["afc_w"].T).astype(f16),
        "w_amv": np.ascontiguousarray(g["am_v_w"].T).astype(f16),
        "w_ama": np.ascontiguousarray(g["am_a_w"].T).astype(f16),
        "w_amo": np.ascontiguousarray(g["am_o_w"].T).astype(f16),
        "w_V": np.ascontiguousarray(g["V_w"].T).astype(f16),
        "w_W": np.ascontiguousarray(g["W_w"].T).astype(f16),
        "w_cls": np.ascontiguousarray(g["cls_w"].T).astype(f32),
        "b_vfc": g["vfc_b"].reshape(128, 1).copy(),
        "b_afc": g["afc_b"].reshape(128, 1).copy(),
        "b_am": (g["am_v_b"] + g["am_a_b"]).reshape(128, 1),
        "b_V": g["V_b"].reshape(HD, 1).copy(),
        "b_amo": np.full((BPC, 1), g["am_o_b"][0], f32),
        "b_cls": g["cls_b"].reshape(2, 1).copy(),
        "ones32": np.ones((1, 128), f32),
    }
    in_maps = []
    for c in range(NCORES):
        m = dict(shared)
        m["vfT"] = np.ascontiguousarray(vfT_all[c * BPC:(c + 1) * BPC])
        m["afT"] = np.ascontiguousarray(afT_all[c * BPC:(c + 1) * BPC])
        in_maps.append(m)
    return in_maps


def kernel(**inputs):
    nc = _get_program()
    in_maps = _prep_inputs(inputs)
    res = run_bass_kernel_spmd(nc, in_maps, list(range(NCORES))).results
    scores = np.concatenate([res[c]["scores"] for c in range(NCORES)], axis=0)
    logits = np.concatenate([res[c]["logits"] for c in range(NCORES)], axis=0)
    return scores.astype(np.float32), logits.astype(np.float32)
